# revision 39
# baseline (speedup 1.0000x reference)
"""GriffinBlock1D Trainium2 Bass kernel.

Sharding: 8 cores = (batch b, T-half). Each core computes the GLRU only on
its own 544-token window plus a 64-token scan warm-up (the linear-recurrence
carry decays ~e^-0.97/step, so the truncated prefix is exact to fp32);
attention + FFN run on the core's own 512-token half (16-token halo,
data-driven masks keep the SPMD program identical on all cores).

Layout: feature-major — activations stored [d (partitions), t (free)].
All weights pre-transposed on host so the contraction dim lands on
partitions. Matmul operands are bf16 (PSUM accumulation stays fp32);
LayerNorm partition-dim reductions run as ones-vector matmuls on the PE;
per-token scale rows are broadcast across partitions on GpSimd.

Host-side algebraic fusions: cand = gv2 @ W_state.T = x @ (W_state@W_in2).T
(one matmul instead of two); ffn LN affine folded into W1/b1.
"""

import numpy as np
import ml_dtypes

import concourse.bass as bass
import concourse.mybir as mybir
import concourse.tile as tile
from concourse import bacc
from concourse.bass_utils import run_bass_kernel_spmd

F32 = mybir.dt.float32
BF16 = mybir.dt.bfloat16
AF = mybir.ActivationFunctionType
ALU = mybir.AluOpType

B, T, D, H, WIN, FFD = 4, 1024, 512, 4, 16, 2048
DH = D // H          # 128
TL = T // 2          # 512 tokens per core
WT = TL + 2 * WIN    # 544-token window (with halo)
WARM = 64            # GLRU scan warm-up tokens (state decays ~e^-0.97/step)
TG = WARM + WT       # 608 local GLRU columns; window = cols [WARM:WARM+WT]
EPS = 1e-5
SCL = 1.0 / np.sqrt(DH)
NCORES = 8

_CACHE = {}


def _shape_act_tables(arch):
    """Steer the act-table chooser to the combined Ln+Exp set.

    insert_act_table_loads picks the FIRST table set containing a needed
    function; 'exp_and_others' / 'natural_log' precede
    'natural_log_exp_and_others', so an Ln->Exp pair loads two tables
    (~1.3us each) instead of one. Emptying the shadowing entries in the
    cached dict (entries kept, so act_func_set_id indices stay valid)
    makes both functions resolve to the combined set.
    """
    from concourse import hw_specs
    try:
        tables = hw_specs.get_activation_tables(arch)
        combined = "natural_log_exp_and_others"
        if combined in tables and tables[combined] >= tables.get("natural_log", set()):
            for name in ("exp_and_others", "natural_log"):
                if name in tables:
                    tables[name] = set()
    except Exception:
        pass


def _build_nc():
    nc = bacc.Bacc("TRN2", target_bir_lowering=False, debug=False)
    _shape_act_tables(nc.m.arch)

    di = lambda n, s, dt=BF16: nc.dram_tensor(n, s, dt, kind="ExternalInput")
    xt_d = di("xt", [D, TG])
    winT_d = di("w_in1T", [D, D])
    wcT_d = di("wcT", [D, D])
    wqT_d = di("wqT", [D, D])
    wkT_d = di("wkT", [D, D])
    wvT_d = di("wvT", [D, D])
    wpT_d = di("wpT", [D, D])
    w1T_d = di("w1T", [D, FFD])
    w2T_d = di("w2T", [FFD, D])
    # packed consts: [p, 16 b1 | 4 b2 | 32 lncol]
    consts_d = di("consts", [128, 52], F32)
    ones_d = di("ones", [128, WT])
    mask12_d = di("mask12", [128, 2, 2, 256])
    mask3_d = di("mask3", [32, 2, 256])
    out_d = nc.dram_tensor("outp", [4, 128, TL], BF16, kind="ExternalOutput")

    with tile.TileContext(nc) as tc:
        with tc.tile_pool(name="cp", bufs=1) as cp, \
             tc.tile_pool(name="sq", bufs=2) as sqp, \
             tc.tile_pool(name="ep", bufs=2) as ep, \
             tc.tile_pool(name="rw", bufs=1) as rw, \
             tc.tile_pool(name="bc", bufs=2) as bc, \
             tc.tile_pool(name="st", bufs=2) as st, \
             tc.tile_pool(name="ps", bufs=2, space="PSUM") as ps:

            # ---------------- weights (all prefetchable at t=0) ------------
            # xt/winT/wcT split per-kk so the first GLRU matmul starts early;
            # consts/masks queued AFTER the weights (SP issues FIFO)
            xt_sb = cp.tile([128, 4, TG], BF16, tag="xt")
            winT_sb = cp.tile([128, 4, D], BF16, tag="winT")
            wcT_sb = cp.tile([128, 4, D], BF16, tag="wst")
            for kk in range(4):
                rs = slice(kk * 128, (kk + 1) * 128)
                nc.sync.dma_start(xt_sb[:, kk, :], xt_d[rs, :])
                nc.sync.dma_start(winT_sb[:, kk, :], winT_d[rs, :])
                nc.sync.dma_start(wcT_sb[:, kk, :], wcT_d[rs, :])
            wqk_sb = cp.tile([128, 8, D], BF16, tag="wqk")
            nc.sync.dma_start(wqk_sb[:, 0:4, :],
                              wqT_d[:, :].rearrange("(a p) e -> p a e", p=128))
            nc.sync.dma_start(wqk_sb[:, 4:8, :],
                              wkT_d[:, :].rearrange("(a p) e -> p a e", p=128))
            wv_sb = cp.tile([128, 4, D], BF16, tag="wv")
            nc.sync.dma_start(wv_sb, wvT_d[:, :].rearrange("(a p) e -> p a e", p=128))
            wp_sb = cp.tile([128, 4, D], BF16, tag="wp")
            nc.sync.dma_start(wp_sb, wpT_d[:, :].rearrange("(a p) e -> p a e", p=128))
            w1_sb = cp.tile([128, 4, FFD], BF16, tag="w1")
            nc.sync.dma_start(w1_sb, w1T_d[:, :].rearrange("(a p) e -> p a e", p=128))
            w2_sb = cp.tile([128, 16, D], BF16, tag="w2")
            nc.sync.dma_start(w2_sb, w2T_d[:, :].rearrange("(a p) e -> p a e", p=128))

            # ---------------- consts (after weights in the DMA queue) ------
            consts_sb = cp.tile([128, 52], F32, tag="consts")
            nc.sync.dma_start(consts_sb, consts_d[:, :])
            b1_sb = consts_sb[:, 0:16]
            b2_sb = consts_sb[:, 16:20]
            # lncol flat layout: 20 + gb*16 + ln_idx*4 + et
            ln_col = lambda gb, ln_idx, et: consts_sb[
                :, 20 + gb * 16 + ln_idx * 4 + et:21 + gb * 16 + ln_idx * 4 + et]
            ones_sb = cp.tile([128, WT], BF16, tag="ones")
            nc.sync.dma_start(ones_sb, ones_d[:, :])
            m12_sb = cp.tile([128, 2, 2, 256], BF16, tag="m12")
            nc.sync.dma_start(m12_sb, mask12_d[:, :, :, :])
            m3_sb = cp.tile([32, 2, 256], BF16, tag="m3")
            nc.sync.dma_start(m3_sb, mask3_d[:, :, :])

            u_t = [cp.tile([128, TG], F32, tag=f"u{i}", name=f"u{i}") for i in range(4)]
            w_t = [cp.tile([128, TG], F32, tag=f"um{i}", name=f"um{i}") for i in range(4)]

            # ---------------- GLRU matmuls + scan (et-major for overlap) ----
            # ywin = y cols [WARM:WARM+WT]; scan warm-up makes the carry exact
            y_t = [cp.tile([128, TG], BF16, tag=f"y{i}", name=f"y{i}") for i in range(4)]
            TC = TG // 2
            for et in range(4):
                for nch in range(2):
                    tsl = slice(nch * TC, (nch + 1) * TC)
                    g1 = ps.tile([128, TC], F32, tag="mm")
                    for kk in range(4):
                        nc.tensor.matmul(
                            g1, winT_sb[:, kk, et * 128:(et + 1) * 128],
                            xt_sb[:, kk, tsl], start=kk == 0, stop=kk == 3)
                    nc.scalar.activation(u_t[et][:, tsl], g1, AF.Sigmoid)
                    nc.vector.tensor_scalar(w_t[et][:, tsl], u_t[et][:, tsl],
                                            -1.0, 1.0, ALU.mult, ALU.add)
                    cd = ps.tile([128, TC], F32, tag="mm")
                    for kk in range(4):
                        nc.tensor.matmul(
                            cd, wcT_sb[:, kk, et * 128:(et + 1) * 128],
                            xt_sb[:, kk, tsl], start=kk == 0, stop=kk == 3)
                    # w = (1 - u) * cand
                    nc.vector.tensor_mul(w_t[et][:, tsl], w_t[et][:, tsl], cd)
                nc.vector.tensor_tensor_scan(y_t[et], u_t[et], w_t[et], 0.0,
                                             ALU.mult, ALU.add)
            ywin = lambda et: y_t[et][:, WARM:WARM + WT]
            # preload the Ln/Exp act table while the PE finishes GLRU
            scr8 = rw.tile([1, 8], F32, tag="scr8")
            nc.scalar.activation(scr8, u_t[0][0:1, 0:8], AF.Exp)

            # ---------------- LayerNorm helper ----------------
            def layer_norm(xin, ln_idx, out_get, ncols, affine=True, chunk=512):
                # xin(et) -> [128, ncols] BF16 AP; out written per chunk
                for c0 in range(0, ncols, chunk):
                    cn = min(chunk, ncols - c0)
                    cs = slice(c0, c0 + cn)
                    s1 = ps.tile([1, cn], F32, tag="aux")
                    s2 = ps.tile([1, cn], F32, tag="aux")
                    for et in range(4):
                        sq = sqp.tile([128, cn], BF16, tag="sq")
                        nc.vector.tensor_mul(sq, xin(et)[:, cs], xin(et)[:, cs])
                        nc.tensor.matmul(s1, ones_sb[:, 0:1], xin(et)[:, cs],
                                         start=et == 0, stop=et == 3)
                        nc.tensor.matmul(s2, ones_sb[:, 0:1], sq,
                                         start=et == 0, stop=et == 3)
                    m = rw.tile([1, cn], F32, tag="m")
                    nc.vector.tensor_scalar(m, s1, 1.0 / D, None, ALU.mult)
                    ve = rw.tile([1, cn], F32, tag="ve")
                    nc.vector.tensor_scalar(ve, s2, 1.0 / D, EPS,
                                            ALU.mult, ALU.add)
                    m2 = rw.tile([1, cn], F32, tag="m2")
                    nc.vector.tensor_mul(m2, m, m)
                    nc.vector.tensor_sub(ve, ve, m2)
                    lnv = rw.tile([1, cn], F32, tag="lnv")
                    nc.scalar.activation(lnv, ve, AF.Ln)
                    rr = rw.tile([1, cn], F32, tag="rr")
                    nc.scalar.activation(rr, lnv, AF.Exp, scale=-0.5)
                    # rows [rstd, -m*rstd] packed -> one partition broadcast
                    rq = rw.tile([1, 2, cn], BF16, tag="rq16")
                    nc.vector.tensor_copy(rq[:, 0, :], rr)
                    nc.vector.scalar_tensor_tensor(rq[:, 1, :], rr, -1.0, m,
                                                   ALU.mult, ALU.mult)
                    rqb = bc.tile([128, 2, cn], BF16, tag="rqb")
                    nc.gpsimd.partition_broadcast(rqb, rq)
                    rb, qb = rqb[:, 0, :], rqb[:, 1, :]
                    for et in range(4):
                        o = out_get(et)[:, cs]
                        nc.vector.tensor_mul(o, xin(et)[:, cs], rb)
                        nc.vector.tensor_add(o, o, qb)
                        if affine:
                            nc.vector.tensor_scalar(
                                o, o, ln_col(0, ln_idx, et),
                                ln_col(1, ln_idx, et), ALU.mult, ALU.add)

            # ---------------- LN1: x1 = LN(ywin) ----------------
            x1 = cp.tile([128, 4, WT], BF16, tag="x1")
            layer_norm(ywin, 0, lambda et: x1[:, et, :], WT)

            # ---------------- attention ----------------
            q_sb = cp.tile([128, 4, TL], BF16, tag="winT")    # per-head q (fm)
            k_sb = cp.tile([128, 4, WT], BF16, tag="xt")      # per-head k (fm)
            for h in range(4):
                qp = ps.tile([128, TL], F32, tag="mm")
                for kk in range(4):
                    nc.tensor.matmul(qp, wqk_sb[:, kk, h * 128:(h + 1) * 128],
                                     x1[:, kk, WIN:WIN + TL],
                                     start=kk == 0, stop=kk == 3)
                nc.vector.tensor_copy(q_sb[:, h, :], qp)
                kp = ps.tile([128, TL], F32, tag="mm")
                kp2 = ps.tile([128, 32], F32, tag="scr3")
                for kk in range(4):
                    nc.tensor.matmul(kp, wqk_sb[:, 4 + kk, h * 128:(h + 1) * 128],
                                     x1[:, kk, 0:TL], start=kk == 0, stop=kk == 3)
                    nc.tensor.matmul(kp2,
                                     wqk_sb[:, 4 + kk, h * 128:(h + 1) * 128],
                                     x1[:, kk, TL:WT],
                                     start=kk == 0, stop=kk == 3)
                nc.vector.tensor_copy(k_sb[:, h, 0:TL], kp)
                nc.vector.tensor_copy(k_sb[:, h, TL:WT], kp2)

            # v token-major: [t' (part), dh_all] in 5 chunks of <=128 t'
            v_sb = cp.tile([128, 5, D], BF16, tag="wst")
            for c in range(5):
                rows = 128 if c < 4 else 32
                vp = ps.tile([128, D], F32, tag="mm")
                for kk in range(4):
                    nc.tensor.matmul(vp[0:rows, :],
                                     x1[:, kk, c * 128:c * 128 + rows],
                                     wv_sb[:, kk, :],
                                     start=kk == 0, stop=kk == 3)
                nc.scalar.activation(v_sb[0:rows, c, :], vp[0:rows, :],
                                     AF.Identity)

            a2 = [cp.tile([128, 2, TL], BF16, tag=f"y{i}", name=f"a2_{i}") for i in range(2)]
            for h in range(4):
                den = ps.tile([1, TL], F32, tag="aux")
                ao = ps.tile([128, TL], F32, tag="mm")
                for j in range(2):
                    jt = slice(256 * j, 256 * j + 256)
                    s12 = ps.tile([128, 512], F32, tag="scr12")
                    s3 = ps.tile([32, 256], F32, tag="scr3")
                    nc.tensor.matmul(s12[:, 0:256],
                                     k_sb[:, h, 256 * j:256 * j + 128],
                                     q_sb[:, h, jt], start=True, stop=True)
                    nc.tensor.matmul(s12[:, 256:512],
                                     k_sb[:, h, 256 * j + 128:256 * j + 256],
                                     q_sb[:, h, jt], start=True, stop=True)
                    nc.tensor.matmul(s3, k_sb[:, h, 256 * j + 256:256 * j + 288],
                                     q_sb[:, h, jt], start=True, stop=True)
                    e1 = ep.tile([128, 256], BF16, tag="e1")
                    nc.scalar.activation(e1, s12[:, 0:256], AF.Exp, scale=SCL)
                    e2 = ep.tile([128, 256], BF16, tag="e2")
                    nc.scalar.activation(e2, s12[:, 256:512], AF.Exp, scale=SCL)
                    e3 = ep.tile([32, 256], BF16, tag="e3")
                    nc.scalar.activation(e3, s3, AF.Exp, scale=SCL)
                    nc.vector.tensor_mul(e1, e1, m12_sb[:, j, 0, :])
                    nc.vector.tensor_mul(e2, e2, m12_sb[:, j, 1, :])
                    nc.vector.tensor_mul(e3, e3, m3_sb[:, j, :])
                    nc.tensor.matmul(den[0:1, jt], ones_sb[:, 0:1],
                                     e1, start=True, stop=False)
                    nc.tensor.matmul(den[0:1, jt], ones_sb[:, 0:1],
                                     e2, start=False, stop=False)
                    nc.tensor.matmul(den[0:1, jt], ones_sb[0:32, 0:1],
                                     e3, start=False, stop=True)
                    hsl = slice(h * 128, (h + 1) * 128)
                    nc.tensor.matmul(ao[:, jt], v_sb[:, 2 * j, hsl],
                                     e1, start=True, stop=False)
                    nc.tensor.matmul(ao[:, jt], v_sb[:, 2 * j + 1, hsl],
                                     e2, start=False, stop=False)
                    nc.tensor.matmul(ao[:, jt], v_sb[0:32, 2 * j + 2, hsl],
                                     e3, start=False, stop=True)
                rec = rw.tile([1, TL], F32, tag="rec")
                nc.vector.reciprocal(rec, den)
                dbc = st.tile([128, TL], F32, tag="dbc")
                nc.gpsimd.partition_broadcast(dbc, rec)
                nc.vector.tensor_mul(a2[h // 2][:, h % 2, :], ao, dbc)

            # ---------------- proj + residual, LN2 ----------------
            x2pre = [cp.tile([128, 2, TL], BF16, tag=f"y{i+2}", name=f"x2pre{i}") for i in range(2)]
            for et in range(4):
                pp = ps.tile([128, TL], F32, tag="mm")
                for kk in range(4):
                    nc.tensor.matmul(pp, wp_sb[:, kk, et * 128:(et + 1) * 128],
                                     a2[kk // 2][:, kk % 2, :],
                                     start=kk == 0, stop=kk == 3)
                nc.vector.tensor_add(x2pre[et // 2][:, et % 2, :],
                                     x1[:, et, WIN:WIN + TL], pp)
            x2 = cp.tile([128, 4, TL], BF16, tag="ywin")
            layer_norm(lambda et: x2pre[et // 2][:, et % 2, :], 1,
                       lambda et: x2[:, et, :], TL, chunk=256)

            # ---------------- FFN (ffn LN affine folded into W1/b1) -------
            xf = cp.tile([128, 4, TL], BF16, tag="x1")
            layer_norm(lambda et: x2[:, et, :], 2, lambda et: xf[:, et, :], TL,
                       affine=False, chunk=256)

            # token-split (tb halves) so x3/LN4/out of tb0 overlap FFN of tb1
            hg = [cp.tile([128, T], BF16, tag=f"u{i}", name=f"hga{i}") for i in range(4)] + \
                 [cp.tile([128, T], BF16, tag=f"um{i}", name=f"hgb{i}") for i in range(4)]
            x3 = cp.tile([128, 4, TL], BF16, tag="winT")
            outt = cp.tile([128, 4, TL], BF16, tag="ywin")
            for ft in range(16):
                hp = ps.tile([128, TL], F32, tag="mm")
                for kk in range(4):
                    nc.tensor.matmul(hp, w1_sb[:, kk, ft * 128:(ft + 1) * 128],
                                     xf[:, kk, :], start=kk == 0, stop=kk == 3)
                nc.scalar.activation(
                    hg[ft // 2][:, (ft % 2) * 512:(ft % 2) * 512 + 512],
                    hp, AF.Gelu, bias=b1_sb[:, ft:ft + 1])
            for tb in range(2):
                ts = slice(tb * 256, tb * 256 + 256)
                ops = [ps.tile([128, 256], F32, tag="mm", name=f"op0_{tb}"),
                       ps.tile([128, 256], F32, tag="mm", name=f"op1_{tb}"),
                       ps.tile([128, 256], F32, tag="scr12", name=f"op2_{tb}"),
                       ps.tile([128, 256], F32, tag="scr12", name=f"op3_{tb}")]
                for kk in range(16):
                    for et in range(4):
                        nc.tensor.matmul(
                            ops[et], w2_sb[:, kk, et * 128:(et + 1) * 128],
                            hg[kk // 2][:, (kk % 2) * 512 + tb * 256:
                                        (kk % 2) * 512 + tb * 256 + 256],
                            start=kk == 0, stop=kk == 15)
                for et in range(4):
                    nc.vector.scalar_tensor_tensor(
                        x3[:, et, ts], ops[et], b2_sb[:, et:et + 1],
                        x2[:, et, ts], ALU.add, ALU.add)
                layer_norm(lambda et: x3[:, et, ts], 3,
                           lambda et: outt[:, et, ts], 256, chunk=256)
                for et in range(4):
                    nc.sync.dma_start(out_d[et, :, ts], outt[:, et, ts])

    nc.compile()
    return nc


def _host_inputs(x, W_in, W_state, glru_g, glru_b, Wq, Wk, Wv, Wp, attn_g,
                 attn_b, ffn_g, ffn_b, W1, b1, W2, b2, out_g, out_b):
    f32 = np.float32
    bf = ml_dtypes.bfloat16
    cb = lambda a: np.ascontiguousarray(np.asarray(a, f32), dtype=None).astype(bf)
    ct = lambda a: np.ascontiguousarray(a, dtype=f32)
    W_in = np.asarray(W_in, f32)
    W_state = np.asarray(W_state, f32)
    Wc = W_state @ W_in[D:]                      # cand = x @ Wc.T
    W1 = np.asarray(W1, f32)
    ffn_g = np.asarray(ffn_g, f32)
    ffn_b = np.asarray(ffn_b, f32)
    W1g = W1 * ffn_g[None, :]
    b1f = np.asarray(b1, f32) + W1 @ ffn_b
    # per-feature LN affine columns: [p, g/b, ln_idx, et]
    gs = np.stack([glru_g, attn_g, np.ones(D, f32), out_g]).astype(f32)
    bs = np.stack([glru_b, attn_b, np.zeros(D, f32), out_b]).astype(f32)
    lncol = np.stack([gs, bs]).reshape(2, 4, 4, 128).transpose(3, 0, 1, 2)
    consts = np.concatenate(
        [b1f.reshape(FFD // 128, 128).T,
         np.asarray(b2, f32).reshape(D // 128, 128).T,
         lncol.reshape(128, 32)], axis=1)
    shared = {
        "w_in1T": cb(W_in[:D].T), "wcT": cb(Wc.T),
        "wqT": cb(np.asarray(Wq, f32).T), "wkT": cb(np.asarray(Wk, f32).T),
        "wvT": cb(np.asarray(Wv, f32).T), "wpT": cb(np.asarray(Wp, f32).T),
        "w1T": cb(W1g.T), "w2T": cb(np.asarray(W2, f32).T),
        "consts": ct(consts),
        "ones": np.ones((128, WT), bf),
    }
    in_maps = []
    xf32 = np.asarray(x, f32)
    for core in range(NCORES):
        b, half = core // 2, core % 2
        h0 = half * TL
        m = dict(shared)
        # local GLRU slab: global cols [h0-WIN-WARM, h0+TL+WIN), zero-padded
        start = h0 - WIN - WARM
        xloc = np.zeros((D, TG), f32)
        lo, hi = max(0, start), min(T, start + TG)
        xloc[:, lo - start:hi - start] = xf32[b].T[:, lo:hi]
        m["xt"] = xloc.astype(bf)
        # masks: window rows r (t' = h0 + 256j + r - 16), cols c (t = h0+256j+c)
        m12 = np.zeros((128, 2, 2, 256), f32)
        m3 = np.zeros((32, 2, 256), f32)
        c = np.arange(256)
        for j in range(2):
            for piece in range(3):
                r = np.arange(128 if piece < 2 else 32) + 128 * piece
                tpg = h0 + 256 * j + r - 16
                band = (np.abs(r[:, None] - c[None, :] - 16) <= 16) \
                    & (tpg[:, None] >= 0) & (tpg[:, None] < T)
                if piece < 2:
                    m12[:, j, piece, :] = band
                else:
                    m3[:, j, :] = band
        m["mask12"], m["mask3"] = m12.astype(bf), m3.astype(bf)
        in_maps.append(m)
    return in_maps


def kernel(**inputs):
    if "nc" not in _CACHE:
        _CACHE["nc"] = _build_nc()
    nc = _CACHE["nc"]
    in_maps = _host_inputs(**inputs)
    res = run_bass_kernel_spmd(nc, in_maps, core_ids=list(range(NCORES)),
                               **_CACHE.get("run_kwargs", {}))
    _CACHE["last_result"] = res
    out = np.empty((B, T, D), np.float32)
    for core in range(NCORES):
        b, half = core // 2, core % 2
        o = res.results[core]["outp"]          # [4, 128, TL]
        out[b, half * TL:(half + 1) * TL, :] = \
            o.reshape(D, TL).T
    return out


# revision 46
# speedup vs baseline: 1.0954x; 1.0954x over previous
"""GriffinBlock1D Trainium2 Bass kernel.

Sharding: 8 cores = (batch b, T-half). Each core computes the GLRU only on
its own 544-token window plus a 64-token scan warm-up (the linear-recurrence
carry decays ~e^-0.97/step, so the truncated prefix is exact to fp32);
attention + FFN run on the core's own 512-token half (16-token halo,
data-driven masks keep the SPMD program identical on all cores).

Layout: feature-major — activations stored [d (partitions), t (free)].
All weights pre-transposed on host so the contraction dim lands on
partitions. Matmul operands are bf16 (PSUM accumulation stays fp32);
LayerNorm partition-dim reductions run as ones-vector matmuls on the PE;
per-token scale rows are broadcast across partitions on GpSimd.

Host-side algebraic fusions: cand = gv2 @ W_state.T = x @ (W_state@W_in2).T
(one matmul instead of two); ffn LN affine folded into W1/b1.
"""

import numpy as np
import ml_dtypes

import concourse.bass as bass
import concourse.mybir as mybir
import concourse.tile as tile
from concourse import bacc
from concourse.bass_utils import run_bass_kernel_spmd

F32 = mybir.dt.float32
BF16 = mybir.dt.bfloat16
AF = mybir.ActivationFunctionType
ALU = mybir.AluOpType

B, T, D, H, WIN, FFD = 4, 1024, 512, 4, 16, 2048
DH = D // H          # 128
TL = T // 2          # 512 tokens per core
WT = TL + 2 * WIN    # 544-token window (with halo)
WARM = 64            # GLRU scan warm-up tokens (state decays ~e^-0.97/step)
TG = WARM + WT       # 608 local GLRU columns; window = cols [WARM:WARM+WT]
EPS = 1e-5
SCL = 1.0 / np.sqrt(DH)
NCORES = 8

_CACHE = {}


def _shape_act_tables(arch):
    """Steer the act-table chooser to the combined Ln+Exp set.

    insert_act_table_loads picks the FIRST table set containing a needed
    function; 'exp_and_others' / 'natural_log' precede
    'natural_log_exp_and_others', so an Ln->Exp pair loads two tables
    (~1.3us each) instead of one. Emptying the shadowing entries in the
    cached dict (entries kept, so act_func_set_id indices stay valid)
    makes both functions resolve to the combined set.
    """
    from concourse import hw_specs
    try:
        tables = hw_specs.get_activation_tables(arch)
        combined = "natural_log_exp_and_others"
        if combined in tables and tables[combined] >= tables.get("natural_log", set()):
            for name in ("exp_and_others", "natural_log"):
                if name in tables:
                    tables[name] = set()
    except Exception:
        pass


def _build_nc():
    nc = bacc.Bacc("TRN2", target_bir_lowering=False, debug=False)
    _shape_act_tables(nc.m.arch)

    di = lambda n, s, dt=BF16: nc.dram_tensor(n, s, dt, kind="ExternalInput")
    xt_d = di("xt", [D, TG])
    winT_d = di("w_in1T", [D, D])
    wcT_d = di("wcT", [D, D])
    wqT_d = di("wqT", [D, D])
    wkT_d = di("wkT", [D, D])
    wvT_d = di("wvT", [D, D])
    wpT_d = di("wpT", [D, D])
    w1T_d = di("w1T", [D, FFD])
    w2T_d = di("w2T", [FFD, D])
    # packed consts: [p, 16 b1 | 4 b2 | 32 lncol]
    consts_d = di("consts", [128, 52], F32)
    ones_d = di("ones", [128, WT])
    mask12_d = di("mask12", [128, 2, 2, 256])
    mask3_d = di("mask3", [32, 2, 256])
    out_d = nc.dram_tensor("outp", [4, 128, TL], BF16, kind="ExternalOutput")

    with tile.TileContext(nc) as tc:
        with tc.tile_pool(name="cp", bufs=1) as cp, \
             tc.tile_pool(name="sq", bufs=2) as sqp, \
             tc.tile_pool(name="ep", bufs=3) as ep, \
             tc.tile_pool(name="rw", bufs=1) as rw, \
             tc.tile_pool(name="bc", bufs=2) as bc, \
             tc.tile_pool(name="st", bufs=2) as st, \
             tc.tile_pool(name="ps", bufs=2, space="PSUM") as ps:

            # ---------------- weights (all prefetchable at t=0) ------------
            # xt/winT/wcT split per-kk so the first GLRU matmul starts early;
            # consts/masks queued AFTER the weights (SP issues FIFO)
            xt_sb = cp.tile([128, 4, TG], BF16, tag="xt")
            winT_sb = cp.tile([128, 4, D], BF16, tag="winT")
            wcT_sb = cp.tile([128, 4, D], BF16, tag="wst")
            for kk in range(4):
                rs = slice(kk * 128, (kk + 1) * 128)
                nc.sync.dma_start(xt_sb[:, kk, :], xt_d[rs, :])
                nc.sync.dma_start(winT_sb[:, kk, :], winT_d[rs, :])
                nc.sync.dma_start(wcT_sb[:, kk, :], wcT_d[rs, :])
            # remaining DMAs ordered by first use (SP issues FIFO, the model
            # serializes the DMA resource): ones/consts (LN1), q/k weights,
            # masks (attention), v/p weights, then the FFN weights
            consts_sb = cp.tile([128, 52], F32, tag="consts")
            nc.sync.dma_start(consts_sb, consts_d[:, :])
            b1_sb = consts_sb[:, 0:16]
            b2_sb = consts_sb[:, 16:20]
            # lncol flat layout: 20 + gb*16 + ln_idx*4 + et
            ln_col = lambda gb, ln_idx, et: consts_sb[
                :, 20 + gb * 16 + ln_idx * 4 + et:21 + gb * 16 + ln_idx * 4 + et]
            ones_sb = cp.tile([128, WT], BF16, tag="ones")
            nc.sync.dma_start(ones_sb, ones_d[:, :])
            wqk_sb = cp.tile([128, 8, D], BF16, tag="wqk")
            nc.sync.dma_start(wqk_sb[:, 0:4, :],
                              wqT_d[:, :].rearrange("(a p) e -> p a e", p=128))
            nc.sync.dma_start(wqk_sb[:, 4:8, :],
                              wkT_d[:, :].rearrange("(a p) e -> p a e", p=128))
            m12_sb = cp.tile([128, 2, 2, 256], BF16, tag="m12")
            nc.sync.dma_start(m12_sb, mask12_d[:, :, :, :])
            m3_sb = cp.tile([32, 2, 256], BF16, tag="m3")
            nc.sync.dma_start(m3_sb, mask3_d[:, :, :])
            wv_sb = cp.tile([128, 4, D], BF16, tag="wv")
            nc.sync.dma_start(wv_sb, wvT_d[:, :].rearrange("(a p) e -> p a e", p=128))
            wp_sb = cp.tile([128, 4, D], BF16, tag="wp")
            nc.sync.dma_start(wp_sb, wpT_d[:, :].rearrange("(a p) e -> p a e", p=128))
            w1_sb = cp.tile([128, 4, FFD], BF16, tag="w1")
            nc.sync.dma_start(w1_sb, w1T_d[:, :].rearrange("(a p) e -> p a e", p=128))
            w2_sb = cp.tile([128, 16, D], BF16, tag="w2")
            nc.sync.dma_start(w2_sb, w2T_d[:, :].rearrange("(a p) e -> p a e", p=128))

            u_t = [cp.tile([128, TG], F32, tag=f"u{i}", name=f"u{i}") for i in range(4)]
            w_t = [cp.tile([128, TG], F32, tag=f"um{i}", name=f"um{i}") for i in range(4)]

            # ---------------- GLRU matmuls + scan (et-major for overlap) ----
            # ywin = y cols [WARM:WARM+WT]; scan warm-up makes the carry exact
            y_t = [cp.tile([128, TG], BF16, tag=f"y{i}", name=f"y{i}") for i in range(4)]
            TC = TG // 2
            for et in range(4):
                for nch in range(2):
                    tsl = slice(nch * TC, (nch + 1) * TC)
                    g1 = ps.tile([128, TC], F32, tag="mm")
                    for kk in range(4):
                        nc.tensor.matmul(
                            g1, winT_sb[:, kk, et * 128:(et + 1) * 128],
                            xt_sb[:, kk, tsl], start=kk == 0, stop=kk == 3)
                    nc.scalar.activation(u_t[et][:, tsl], g1, AF.Sigmoid)
                    nc.vector.tensor_scalar(w_t[et][:, tsl], u_t[et][:, tsl],
                                            -1.0, 1.0, ALU.mult, ALU.add)
                    cd = ps.tile([128, TC], F32, tag="mm")
                    for kk in range(4):
                        nc.tensor.matmul(
                            cd, wcT_sb[:, kk, et * 128:(et + 1) * 128],
                            xt_sb[:, kk, tsl], start=kk == 0, stop=kk == 3)
                    # w = (1 - u) * cand
                    nc.vector.tensor_mul(w_t[et][:, tsl], w_t[et][:, tsl], cd)
                nc.vector.tensor_tensor_scan(y_t[et], u_t[et], w_t[et], 0.0,
                                             ALU.mult, ALU.add)
            ywin = lambda et: y_t[et][:, WARM:WARM + WT]
            # preload the Ln/Exp act table while the PE finishes GLRU
            scr8 = rw.tile([1, 8], F32, tag="scr8")
            nc.scalar.activation(scr8, u_t[0][0:1, 0:8], AF.Exp)

            # ---------------- LayerNorm helper ----------------
            def layer_norm(xin, ln_idx, out_get, ncols, affine=True, chunk=512,
                           chunk_order=None):
                # xin(et) -> [128, ncols] BF16 AP; out written per chunk
                starts = chunk_order or list(range(0, ncols, chunk))
                for c0 in starts:
                    cn = min(chunk, ncols - c0)
                    cs = slice(c0, c0 + cn)
                    s1 = ps.tile([1, cn], F32, tag="aux")
                    s2 = ps.tile([1, cn], F32, tag="aux")
                    for et in range(4):
                        sq = sqp.tile([128, cn], BF16, tag="sq")
                        nc.vector.tensor_mul(sq, xin(et)[:, cs], xin(et)[:, cs])
                        nc.tensor.matmul(s1, ones_sb[:, 0:1], xin(et)[:, cs],
                                         start=et == 0, stop=et == 3)
                        nc.tensor.matmul(s2, ones_sb[:, 0:1], sq,
                                         start=et == 0, stop=et == 3)
                    m = rw.tile([1, cn], F32, tag="m")
                    nc.vector.tensor_scalar(m, s1, 1.0 / D, None, ALU.mult)
                    ve = rw.tile([1, cn], F32, tag="ve")
                    nc.vector.tensor_scalar(ve, s2, 1.0 / D, EPS,
                                            ALU.mult, ALU.add)
                    m2 = rw.tile([1, cn], F32, tag="m2")
                    nc.vector.tensor_mul(m2, m, m)
                    nc.vector.tensor_sub(ve, ve, m2)
                    lnv = rw.tile([1, cn], F32, tag="lnv")
                    nc.scalar.activation(lnv, ve, AF.Ln)
                    rr = rw.tile([1, cn], F32, tag="rr")
                    nc.scalar.activation(rr, lnv, AF.Exp, scale=-0.5)
                    # rows [rstd, -m*rstd] packed -> one partition broadcast
                    rq = rw.tile([1, 2, cn], BF16, tag="rq16")
                    nc.vector.tensor_copy(rq[:, 0, :], rr)
                    nc.vector.scalar_tensor_tensor(rq[:, 1, :], rr, -1.0, m,
                                                   ALU.mult, ALU.mult)
                    rqb = bc.tile([128, 2, cn], BF16, tag="rqb")
                    nc.gpsimd.partition_broadcast(rqb[:, 0, :], rq[:, 0, :])
                    nc.gpsimd.partition_broadcast(rqb[:, 1, :], rq[:, 1, :])
                    rb, qb = rqb[:, 0, :], rqb[:, 1, :]
                    for et in range(4):
                        o = out_get(et)[:, cs]
                        nc.vector.tensor_mul(o, xin(et)[:, cs], rb)
                        nc.vector.tensor_add(o, o, qb)
                        if affine:
                            nc.vector.tensor_scalar(
                                o, o, ln_col(0, ln_idx, et),
                                ln_col(1, ln_idx, et), ALU.mult, ALU.add)

            # ---------------- LN1: x1 = LN(ywin) ----------------
            # 32-col tail chunk first: q/k need all chunks, so the small
            # chunk's serial chain hides under the big chunk's stats
            x1 = cp.tile([128, 4, WT], BF16, tag="x1")
            layer_norm(ywin, 0, lambda et: x1[:, et, :], WT,
                       chunk_order=[512, 0])

            # ---------------- attention ----------------
            q_sb = cp.tile([128, 4, TL], BF16, tag="winT")    # per-head q (fm)
            k_sb = cp.tile([128, 4, WT], BF16, tag="xt")      # per-head k (fm)
            for h in range(4):
                qp = ps.tile([128, TL], F32, tag="mm")
                for kk in range(4):
                    nc.tensor.matmul(qp, wqk_sb[:, kk, h * 128:(h + 1) * 128],
                                     x1[:, kk, WIN:WIN + TL],
                                     start=kk == 0, stop=kk == 3)
                nc.vector.tensor_copy(q_sb[:, h, :], qp)
                kp = ps.tile([128, TL], F32, tag="mm")
                kp2 = ps.tile([128, 32], F32, tag="scr3")
                for kk in range(4):
                    nc.tensor.matmul(kp, wqk_sb[:, 4 + kk, h * 128:(h + 1) * 128],
                                     x1[:, kk, 0:TL], start=kk == 0, stop=kk == 3)
                    nc.tensor.matmul(kp2,
                                     wqk_sb[:, 4 + kk, h * 128:(h + 1) * 128],
                                     x1[:, kk, TL:WT],
                                     start=kk == 0, stop=kk == 3)
                nc.vector.tensor_copy(k_sb[:, h, 0:TL], kp)
                nc.vector.tensor_copy(k_sb[:, h, TL:WT], kp2)

            # v token-major: [t' (part), dh_all] in 5 chunks of <=128 t'
            v_sb = cp.tile([128, 5, D], BF16, tag="wst")
            for c in range(5):
                rows = 128 if c < 4 else 32
                vp = ps.tile([128, D], F32, tag="mm")
                for kk in range(4):
                    nc.tensor.matmul(vp[0:rows, :],
                                     x1[:, kk, c * 128:c * 128 + rows],
                                     wv_sb[:, kk, :],
                                     start=kk == 0, stop=kk == 3)
                nc.scalar.activation(v_sb[0:rows, c, :], vp[0:rows, :],
                                     AF.Identity)

            a2 = [cp.tile([128, 2, TL], BF16, tag=f"y{i}", name=f"a2_{i}") for i in range(2)]
            for h in range(4):
                den = ps.tile([1, TL], F32, tag="aux")
                ao = ps.tile([128, TL], F32, tag="mm")
                for j in range(2):
                    jt = slice(256 * j, 256 * j + 256)
                    s12 = ps.tile([128, 512], F32, tag="scr12")
                    s3 = ps.tile([32, 256], F32, tag="scr3")
                    nc.tensor.matmul(s12[:, 0:256],
                                     k_sb[:, h, 256 * j:256 * j + 128],
                                     q_sb[:, h, jt], start=True, stop=True)
                    nc.tensor.matmul(s12[:, 256:512],
                                     k_sb[:, h, 256 * j + 128:256 * j + 256],
                                     q_sb[:, h, jt], start=True, stop=True)
                    nc.tensor.matmul(s3, k_sb[:, h, 256 * j + 256:256 * j + 288],
                                     q_sb[:, h, jt], start=True, stop=True)
                    e1 = ep.tile([128, 256], BF16, tag="e1")
                    nc.scalar.activation(e1, s12[:, 0:256], AF.Exp, scale=SCL)
                    e2 = ep.tile([128, 256], BF16, tag="e2")
                    nc.scalar.activation(e2, s12[:, 256:512], AF.Exp, scale=SCL)
                    e3 = ep.tile([32, 256], BF16, tag="e3")
                    nc.scalar.activation(e3, s3, AF.Exp, scale=SCL)
                    nc.vector.tensor_mul(e1, e1, m12_sb[:, j, 0, :])
                    nc.vector.tensor_mul(e2, e2, m12_sb[:, j, 1, :])
                    nc.vector.tensor_mul(e3, e3, m3_sb[:, j, :])
                    nc.tensor.matmul(den[0:1, jt], ones_sb[:, 0:1],
                                     e1, start=True, stop=False)
                    nc.tensor.matmul(den[0:1, jt], ones_sb[:, 0:1],
                                     e2, start=False, stop=False)
                    nc.tensor.matmul(den[0:1, jt], ones_sb[0:32, 0:1],
                                     e3, start=False, stop=True)
                    hsl = slice(h * 128, (h + 1) * 128)
                    nc.tensor.matmul(ao[:, jt], v_sb[:, 2 * j, hsl],
                                     e1, start=True, stop=False)
                    nc.tensor.matmul(ao[:, jt], v_sb[:, 2 * j + 1, hsl],
                                     e2, start=False, stop=False)
                    nc.tensor.matmul(ao[:, jt], v_sb[0:32, 2 * j + 2, hsl],
                                     e3, start=False, stop=True)
                rec = rw.tile([1, TL], F32, tag="rec")
                nc.vector.reciprocal(rec, den)
                dbc = st.tile([128, TL], F32, tag="dbc")
                nc.gpsimd.partition_broadcast(dbc, rec)
                nc.vector.tensor_mul(a2[h // 2][:, h % 2, :], ao, dbc)

            # ---------------- proj + residual, LN2 ----------------
            x2pre = [cp.tile([128, 2, TL], BF16, tag=f"y{i+2}", name=f"x2pre{i}") for i in range(2)]
            for et in range(4):
                pp = ps.tile([128, TL], F32, tag="mm")
                for kk in range(4):
                    nc.tensor.matmul(pp, wp_sb[:, kk, et * 128:(et + 1) * 128],
                                     a2[kk // 2][:, kk % 2, :],
                                     start=kk == 0, stop=kk == 3)
                nc.vector.tensor_add(x2pre[et // 2][:, et % 2, :],
                                     x1[:, et, WIN:WIN + TL], pp)
            x2 = cp.tile([128, 4, TL], BF16, tag="ywin")
            layer_norm(lambda et: x2pre[et // 2][:, et % 2, :], 1,
                       lambda et: x2[:, et, :], TL, chunk=256)

            # ---------------- FFN (ffn LN affine folded into W1/b1) -------
            xf = cp.tile([128, 4, TL], BF16, tag="x1")
            layer_norm(lambda et: x2[:, et, :], 2, lambda et: xf[:, et, :], TL,
                       affine=False, chunk=256)

            # token-split (tb halves) so x3/LN4/out of tb0 overlap FFN of tb1
            hg = [cp.tile([128, T], BF16, tag=f"u{i}", name=f"hga{i}") for i in range(4)] + \
                 [cp.tile([128, T], BF16, tag=f"um{i}", name=f"hgb{i}") for i in range(4)]
            x3 = cp.tile([128, 4, TL], BF16, tag="winT")
            outt = cp.tile([128, 4, TL], BF16, tag="ywin")
            for ft in range(16):
                hp = ps.tile([128, TL], F32, tag="mm")
                for kk in range(4):
                    nc.tensor.matmul(hp, w1_sb[:, kk, ft * 128:(ft + 1) * 128],
                                     xf[:, kk, :], start=kk == 0, stop=kk == 3)
                nc.scalar.activation(
                    hg[ft // 2][:, (ft % 2) * 512:(ft % 2) * 512 + 512],
                    hp, AF.Gelu, bias=b1_sb[:, ft:ft + 1])
            for tb in range(2):
                ts = slice(tb * 256, tb * 256 + 256)
                ops = [ps.tile([128, 256], F32, tag="mm", name=f"op0_{tb}"),
                       ps.tile([128, 256], F32, tag="mm", name=f"op1_{tb}"),
                       ps.tile([128, 256], F32, tag="scr12", name=f"op2_{tb}"),
                       ps.tile([128, 256], F32, tag="scr12", name=f"op3_{tb}")]
                for kk in range(16):
                    for et in range(4):
                        nc.tensor.matmul(
                            ops[et], w2_sb[:, kk, et * 128:(et + 1) * 128],
                            hg[kk // 2][:, (kk % 2) * 512 + tb * 256:
                                        (kk % 2) * 512 + tb * 256 + 256],
                            start=kk == 0, stop=kk == 15)
                for et in range(4):
                    nc.vector.scalar_tensor_tensor(
                        x3[:, et, ts], ops[et], b2_sb[:, et:et + 1],
                        x2[:, et, ts], ALU.add, ALU.add)
                layer_norm(lambda et: x3[:, et, ts], 3,
                           lambda et: outt[:, et, ts], 256, chunk=256)
                for et in range(4):
                    nc.sync.dma_start(out_d[et, :, ts], outt[:, et, ts])

    nc.compile()
    return nc


def _host_inputs(x, W_in, W_state, glru_g, glru_b, Wq, Wk, Wv, Wp, attn_g,
                 attn_b, ffn_g, ffn_b, W1, b1, W2, b2, out_g, out_b):
    f32 = np.float32
    bf = ml_dtypes.bfloat16
    cb = lambda a: np.ascontiguousarray(np.asarray(a, f32), dtype=None).astype(bf)
    ct = lambda a: np.ascontiguousarray(a, dtype=f32)
    W_in = np.asarray(W_in, f32)
    W_state = np.asarray(W_state, f32)
    Wc = W_state @ W_in[D:]                      # cand = x @ Wc.T
    W1 = np.asarray(W1, f32)
    ffn_g = np.asarray(ffn_g, f32)
    ffn_b = np.asarray(ffn_b, f32)
    W1g = W1 * ffn_g[None, :]
    b1f = np.asarray(b1, f32) + W1 @ ffn_b
    # per-feature LN affine columns: [p, g/b, ln_idx, et]
    gs = np.stack([glru_g, attn_g, np.ones(D, f32), out_g]).astype(f32)
    bs = np.stack([glru_b, attn_b, np.zeros(D, f32), out_b]).astype(f32)
    lncol = np.stack([gs, bs]).reshape(2, 4, 4, 128).transpose(3, 0, 1, 2)
    consts = np.concatenate(
        [b1f.reshape(FFD // 128, 128).T,
         np.asarray(b2, f32).reshape(D // 128, 128).T,
         lncol.reshape(128, 32)], axis=1)
    shared = {
        "w_in1T": cb(W_in[:D].T), "wcT": cb(Wc.T),
        "wqT": cb(np.asarray(Wq, f32).T), "wkT": cb(np.asarray(Wk, f32).T),
        "wvT": cb(np.asarray(Wv, f32).T), "wpT": cb(np.asarray(Wp, f32).T),
        "w1T": cb(W1g.T), "w2T": cb(np.asarray(W2, f32).T),
        "consts": ct(consts),
        "ones": np.ones((128, WT), bf),
    }
    in_maps = []
    xf32 = np.asarray(x, f32)
    for core in range(NCORES):
        b, half = core // 2, core % 2
        h0 = half * TL
        m = dict(shared)
        # local GLRU slab: global cols [h0-WIN-WARM, h0+TL+WIN), zero-padded
        start = h0 - WIN - WARM
        xloc = np.zeros((D, TG), f32)
        lo, hi = max(0, start), min(T, start + TG)
        xloc[:, lo - start:hi - start] = xf32[b].T[:, lo:hi]
        m["xt"] = xloc.astype(bf)
        # masks: window rows r (t' = h0 + 256j + r - 16), cols c (t = h0+256j+c)
        m12 = np.zeros((128, 2, 2, 256), f32)
        m3 = np.zeros((32, 2, 256), f32)
        c = np.arange(256)
        for j in range(2):
            for piece in range(3):
                r = np.arange(128 if piece < 2 else 32) + 128 * piece
                tpg = h0 + 256 * j + r - 16
                band = (np.abs(r[:, None] - c[None, :] - 16) <= 16) \
                    & (tpg[:, None] >= 0) & (tpg[:, None] < T)
                if piece < 2:
                    m12[:, j, piece, :] = band
                else:
                    m3[:, j, :] = band
        m["mask12"], m["mask3"] = m12.astype(bf), m3.astype(bf)
        in_maps.append(m)
    return in_maps


def kernel(**inputs):
    if "nc" not in _CACHE:
        _CACHE["nc"] = _build_nc()
    nc = _CACHE["nc"]
    in_maps = _host_inputs(**inputs)
    res = run_bass_kernel_spmd(nc, in_maps, core_ids=list(range(NCORES)),
                               **_CACHE.get("run_kwargs", {}))
    _CACHE["last_result"] = res
    out = np.empty((B, T, D), np.float32)
    for core in range(NCORES):
        b, half = core // 2, core % 2
        o = res.results[core]["outp"]          # [4, 128, TL]
        out[b, half * TL:(half + 1) * TL, :] = \
            o.reshape(D, TL).T
    return out


# revision 58
# speedup vs baseline: 1.0994x; 1.0036x over previous
"""GriffinBlock1D Trainium2 Bass kernel.

Sharding: 8 cores = (batch b, T-half). Each core computes the GLRU only on
its own 544-token window plus a 64-token scan warm-up (the linear-recurrence
carry decays ~e^-0.97/step, so the truncated prefix is exact to fp32);
attention + FFN run on the core's own 512-token half (16-token halo,
data-driven masks keep the SPMD program identical on all cores).

Layout: feature-major — activations stored [d (partitions), t (free)].
All weights pre-transposed on host so the contraction dim lands on
partitions. Matmul operands are bf16 (PSUM accumulation stays fp32);
LayerNorm partition-dim reductions run as ones-vector matmuls on the PE;
per-token scale rows are broadcast across partitions on GpSimd.

Host-side algebraic fusions: cand = gv2 @ W_state.T = x @ (W_state@W_in2).T
(one matmul instead of two); ffn LN affine folded into W1/b1.
"""

import numpy as np
import ml_dtypes

import concourse.bass as bass
import concourse.mybir as mybir
import concourse.tile as tile
from concourse import bacc
from concourse.bass_utils import run_bass_kernel_spmd

F32 = mybir.dt.float32
BF16 = mybir.dt.bfloat16
AF = mybir.ActivationFunctionType
ALU = mybir.AluOpType

B, T, D, H, WIN, FFD = 4, 1024, 512, 4, 16, 2048
DH = D // H          # 128
TL = T // 2          # 512 tokens per core
WT = TL + 2 * WIN    # 544-token window (with halo)
WARM = 64            # GLRU scan warm-up tokens (state decays ~e^-0.97/step)
TG = WARM + WT       # 608 local GLRU columns; window = cols [WARM:WARM+WT]
EPS = 1e-5
SCL = 1.0 / np.sqrt(DH)
NCORES = 8

_CACHE = {}


def _shape_act_tables(arch):
    """Steer the act-table chooser to the combined Ln+Exp set.

    insert_act_table_loads picks the FIRST table set containing a needed
    function; 'exp_and_others' / 'natural_log' precede
    'natural_log_exp_and_others', so an Ln->Exp pair loads two tables
    (~1.3us each) instead of one. Emptying the shadowing entries in the
    cached dict (entries kept, so act_func_set_id indices stay valid)
    makes both functions resolve to the combined set.
    """
    from concourse import hw_specs
    try:
        tables = hw_specs.get_activation_tables(arch)
        combined = "natural_log_exp_and_others"
        if combined in tables and tables[combined] >= tables.get("natural_log", set()):
            for name in ("exp_and_others", "natural_log"):
                if name in tables:
                    tables[name] = set()
    except Exception:
        pass


def _build_nc():
    nc = bacc.Bacc("TRN2", target_bir_lowering=False, debug=False)
    _shape_act_tables(nc.m.arch)

    di = lambda n, s, dt=BF16: nc.dram_tensor(n, s, dt, kind="ExternalInput")
    xt_d = di("xt", [D, TG])
    winT_d = di("w_in1T", [D, D])
    wcT_d = di("wcT", [D, D])
    wqT_d = di("wqT", [D, D])
    wkT_d = di("wkT", [D, D])
    wvT_d = di("wvT", [D, D])
    wpT_d = di("wpT", [D, D])
    w1T_d = di("w1T", [D, FFD])
    w2T_d = di("w2T", [FFD, D])
    # packed consts: [p, 16 b1 | 4 b2 | 32 lncol]
    consts_d = di("consts", [128, 52], F32)
    ones_d = di("ones", [128, WT])
    mask12_d = di("mask12", [128, 2, 2, 256])
    mask3_d = di("mask3", [32, 2, 256])
    out_d = nc.dram_tensor("outp", [128, 4, TL], BF16, kind="ExternalOutput")

    with tile.TileContext(nc) as tc:
        with tc.tile_pool(name="cp", bufs=1) as cp, \
             tc.tile_pool(name="sq", bufs=2) as sqp, \
             tc.tile_pool(name="ep", bufs=3) as ep, \
             tc.tile_pool(name="rw", bufs=1) as rw, \
             tc.tile_pool(name="bc", bufs=2) as bc, \
             tc.tile_pool(name="st", bufs=2) as st, \
             tc.tile_pool(name="ps", bufs=2, space="PSUM") as ps:

            # ---------------- weights (all prefetchable at t=0) ------------
            # xt/winT/wcT split per-kk so the first GLRU matmul starts early;
            # consts/masks queued AFTER the weights (SP issues FIFO)
            xt_sb = cp.tile([128, 4, TG], BF16, tag="xt")
            winT_sb = cp.tile([128, 4, D], BF16, tag="winT")
            wcT_sb = cp.tile([128, 4, D], BF16, tag="wst")
            for kk in range(4):
                rs = slice(kk * 128, (kk + 1) * 128)
                nc.sync.dma_start(xt_sb[:, kk, :], xt_d[rs, :])
                nc.sync.dma_start(winT_sb[:, kk, :], winT_d[rs, :])
                nc.sync.dma_start(wcT_sb[:, kk, :], wcT_d[rs, :])
            # remaining DMAs ordered by first use (SP issues FIFO, the model
            # serializes the DMA resource): ones/consts (LN1), q/k weights,
            # masks (attention), v/p weights, then the FFN weights
            consts_sb = cp.tile([128, 52], F32, tag="consts")
            nc.sync.dma_start(consts_sb, consts_d[:, :])
            b1_sb = consts_sb[:, 0:16]
            b2_sb = consts_sb[:, 16:20]
            # lncol flat layout: 20 + gb*16 + ln_idx*4 + et
            ln_col = lambda gb, ln_idx, et: consts_sb[
                :, 20 + gb * 16 + ln_idx * 4 + et:21 + gb * 16 + ln_idx * 4 + et]
            ones_sb = cp.tile([128, WT], BF16, tag="ones")
            nc.sync.dma_start(ones_sb, ones_d[:, :])
            wqk_sb = cp.tile([128, 8, D], BF16, tag="wqk")
            nc.sync.dma_start(wqk_sb[:, 0:4, :],
                              wqT_d[:, :].rearrange("(a p) e -> p a e", p=128))
            nc.sync.dma_start(wqk_sb[:, 4:8, :],
                              wkT_d[:, :].rearrange("(a p) e -> p a e", p=128))
            m12_sb = cp.tile([128, 2, 2, 256], BF16, tag="m12")
            nc.sync.dma_start(m12_sb, mask12_d[:, :, :, :])
            m3_sb = cp.tile([32, 2, 256], BF16, tag="m3")
            nc.sync.dma_start(m3_sb, mask3_d[:, :, :])
            wv_sb = cp.tile([128, 4, D], BF16, tag="wv")
            nc.sync.dma_start(wv_sb, wvT_d[:, :].rearrange("(a p) e -> p a e", p=128))
            wp_sb = cp.tile([128, 4, D], BF16, tag="wp")
            nc.sync.dma_start(wp_sb, wpT_d[:, :].rearrange("(a p) e -> p a e", p=128))
            w1_sb = cp.tile([128, 4, FFD], BF16, tag="w1")
            nc.sync.dma_start(w1_sb, w1T_d[:, :].rearrange("(a p) e -> p a e", p=128))
            w2_sb = cp.tile([128, 16, D], BF16, tag="w2")
            nc.sync.dma_start(w2_sb, w2T_d[:, :].rearrange("(a p) e -> p a e", p=128))

            u_t = [cp.tile([128, TG], F32, tag=f"u{i}", name=f"u{i}") for i in range(4)]
            w_t = [cp.tile([128, TG], F32, tag=f"um{i}", name=f"um{i}") for i in range(4)]

            # ---------------- GLRU matmuls + scan (et-major for overlap) ----
            # ywin = y cols [WARM:WARM+WT]; scan warm-up makes the carry exact
            y_t = [cp.tile([128, TG], BF16, tag=f"y{i}", name=f"y{i}") for i in range(4)]
            TC = TG // 2
            for et in range(4):
                for nch in range(2):
                    tsl = slice(nch * TC, (nch + 1) * TC)
                    g1 = ps.tile([128, TC], F32, tag="mm")
                    for kk in range(4):
                        nc.tensor.matmul(
                            g1, winT_sb[:, kk, et * 128:(et + 1) * 128],
                            xt_sb[:, kk, tsl], start=kk == 0, stop=kk == 3)
                    nc.scalar.activation(u_t[et][:, tsl], g1, AF.Sigmoid)
                    nc.vector.tensor_scalar(w_t[et][:, tsl], u_t[et][:, tsl],
                                            -1.0, 1.0, ALU.mult, ALU.add)
                    cd = ps.tile([128, TC], F32, tag="mm")
                    for kk in range(4):
                        nc.tensor.matmul(
                            cd, wcT_sb[:, kk, et * 128:(et + 1) * 128],
                            xt_sb[:, kk, tsl], start=kk == 0, stop=kk == 3)
                    # w = (1 - u) * cand
                    nc.vector.tensor_mul(w_t[et][:, tsl], w_t[et][:, tsl], cd)
                nc.vector.tensor_tensor_scan(y_t[et], u_t[et], w_t[et], 0.0,
                                             ALU.mult, ALU.add)
            ywin = lambda et: y_t[et][:, WARM:WARM + WT]
            # preload the Ln/Exp act table while the PE finishes GLRU
            scr8 = rw.tile([1, 8], F32, tag="scr8")
            nc.scalar.activation(scr8, u_t[0][0:1, 0:8], AF.Exp)

            # ---------------- LayerNorm helper ----------------
            def layer_norm(xin, ln_idx, out_get, ncols, affine=True, chunk=512,
                           chunk_order=None):
                # xin(et) -> [128, ncols] BF16 AP; out written per chunk
                starts = chunk_order or list(range(0, ncols, chunk))
                for c0 in starts:
                    cn = min(chunk, ncols - c0)
                    cs = slice(c0, c0 + cn)
                    s1 = ps.tile([1, cn], F32, tag="aux")
                    s2 = ps.tile([1, cn], F32, tag="aux")
                    for et in range(4):
                        sq = sqp.tile([128, cn], BF16, tag="sq")
                        nc.vector.tensor_mul(sq, xin(et)[:, cs], xin(et)[:, cs])
                        nc.tensor.matmul(s1, ones_sb[:, 0:1], xin(et)[:, cs],
                                         start=et == 0, stop=et == 3)
                        nc.tensor.matmul(s2, ones_sb[:, 0:1], sq,
                                         start=et == 0, stop=et == 3)
                    m = rw.tile([1, cn], F32, tag="m")
                    nc.vector.tensor_scalar(m, s1, 1.0 / D, None, ALU.mult)
                    ve = rw.tile([1, cn], F32, tag="ve")
                    nc.vector.tensor_scalar(ve, s2, 1.0 / D, EPS,
                                            ALU.mult, ALU.add)
                    m2 = rw.tile([1, cn], F32, tag="m2")
                    nc.vector.tensor_mul(m2, m, m)
                    nc.vector.tensor_sub(ve, ve, m2)
                    lnv = rw.tile([1, cn], F32, tag="lnv")
                    nc.scalar.activation(lnv, ve, AF.Ln)
                    rr = rw.tile([1, cn], F32, tag="rr")
                    nc.scalar.activation(rr, lnv, AF.Exp, scale=-0.5)
                    # rows [rstd, -m*rstd] packed -> one partition broadcast
                    rq = rw.tile([1, 2, cn], BF16, tag="rq16")
                    nc.vector.tensor_copy(rq[:, 0, :], rr)
                    nc.vector.scalar_tensor_tensor(rq[:, 1, :], rr, -1.0, m,
                                                   ALU.mult, ALU.mult)
                    rqb = bc.tile([128, 2, cn], BF16, tag="rqb")
                    nc.gpsimd.partition_broadcast(rqb[:, 0, :], rq[:, 0, :])
                    nc.gpsimd.partition_broadcast(rqb[:, 1, :], rq[:, 1, :])
                    rb, qb = rqb[:, 0, :], rqb[:, 1, :]
                    for et in range(4):
                        o = out_get(et)[:, cs]
                        nc.vector.tensor_mul(o, xin(et)[:, cs], rb)
                        nc.vector.tensor_add(o, o, qb)
                        if affine:
                            nc.vector.tensor_scalar(
                                o, o, ln_col(0, ln_idx, et),
                                ln_col(1, ln_idx, et), ALU.mult, ALU.add)

            # ---------------- LN1: x1 = LN(ywin) ----------------
            # 32-col tail chunk first: q/k need all chunks, so the small
            # chunk's serial chain hides under the big chunk's stats
            x1 = cp.tile([128, 4, WT], BF16, tag="x1")
            layer_norm(ywin, 0, lambda et: x1[:, et, :], WT,
                       chunk_order=[512, 0])

            # ---------------- attention ----------------
            q_sb = cp.tile([128, 4, TL], BF16, tag="winT")    # per-head q (fm)
            k_sb = cp.tile([128, 4, WT], BF16, tag="xt")      # per-head k (fm)
            for h in range(4):
                qp = ps.tile([128, TL], F32, tag="mm")
                for kk in range(4):
                    nc.tensor.matmul(qp, wqk_sb[:, kk, h * 128:(h + 1) * 128],
                                     x1[:, kk, WIN:WIN + TL],
                                     start=kk == 0, stop=kk == 3)
                nc.vector.tensor_copy(q_sb[:, h, :], qp)
                kp = ps.tile([128, TL], F32, tag="mm")
                kp2 = ps.tile([128, 32], F32, tag="scr3")
                for kk in range(4):
                    nc.tensor.matmul(kp, wqk_sb[:, 4 + kk, h * 128:(h + 1) * 128],
                                     x1[:, kk, 0:TL], start=kk == 0, stop=kk == 3)
                    nc.tensor.matmul(kp2,
                                     wqk_sb[:, 4 + kk, h * 128:(h + 1) * 128],
                                     x1[:, kk, TL:WT],
                                     start=kk == 0, stop=kk == 3)
                nc.vector.tensor_copy(k_sb[:, h, 0:TL], kp)
                nc.vector.tensor_copy(k_sb[:, h, TL:WT], kp2)

            # v token-major: [t' (part), dh_all] in 5 chunks of <=128 t'
            v_sb = cp.tile([128, 5, D], BF16, tag="wst")
            for c in range(5):
                rows = 128 if c < 4 else 32
                vp = ps.tile([128, D], F32, tag="mm")
                for kk in range(4):
                    nc.tensor.matmul(vp[0:rows, :],
                                     x1[:, kk, c * 128:c * 128 + rows],
                                     wv_sb[:, kk, :],
                                     start=kk == 0, stop=kk == 3)
                nc.scalar.activation(v_sb[0:rows, c, :], vp[0:rows, :],
                                     AF.Identity)

            a2 = [cp.tile([128, 2, TL], BF16, tag=f"y{i}", name=f"a2_{i}") for i in range(2)]
            for h in range(4):
                den = ps.tile([1, TL], F32, tag="aux")
                ao = ps.tile([128, TL], F32, tag="mm")
                for j in range(2):
                    jt = slice(256 * j, 256 * j + 256)
                    s12 = ps.tile([128, 512], F32, tag="scr12")
                    s3 = ps.tile([32, 256], F32, tag="scr3")
                    nc.tensor.matmul(s12[:, 0:256],
                                     k_sb[:, h, 256 * j:256 * j + 128],
                                     q_sb[:, h, jt], start=True, stop=True)
                    nc.tensor.matmul(s12[:, 256:512],
                                     k_sb[:, h, 256 * j + 128:256 * j + 256],
                                     q_sb[:, h, jt], start=True, stop=True)
                    nc.tensor.matmul(s3, k_sb[:, h, 256 * j + 256:256 * j + 288],
                                     q_sb[:, h, jt], start=True, stop=True)
                    e1 = ep.tile([128, 256], BF16, tag="e1")
                    nc.scalar.activation(e1, s12[:, 0:256], AF.Exp, scale=SCL)
                    e2 = ep.tile([128, 256], BF16, tag="e2")
                    nc.scalar.activation(e2, s12[:, 256:512], AF.Exp, scale=SCL)
                    e3 = ep.tile([32, 256], BF16, tag="e3")
                    nc.scalar.activation(e3, s3, AF.Exp, scale=SCL)
                    nc.vector.tensor_mul(e1, e1, m12_sb[:, j, 0, :])
                    nc.vector.tensor_mul(e2, e2, m12_sb[:, j, 1, :])
                    nc.vector.tensor_mul(e3, e3, m3_sb[:, j, :])
                    nc.tensor.matmul(den[0:1, jt], ones_sb[:, 0:1],
                                     e1, start=True, stop=False)
                    nc.tensor.matmul(den[0:1, jt], ones_sb[:, 0:1],
                                     e2, start=False, stop=False)
                    nc.tensor.matmul(den[0:1, jt], ones_sb[0:32, 0:1],
                                     e3, start=False, stop=True)
                    hsl = slice(h * 128, (h + 1) * 128)
                    nc.tensor.matmul(ao[:, jt], v_sb[:, 2 * j, hsl],
                                     e1, start=True, stop=False)
                    nc.tensor.matmul(ao[:, jt], v_sb[:, 2 * j + 1, hsl],
                                     e2, start=False, stop=False)
                    nc.tensor.matmul(ao[:, jt], v_sb[0:32, 2 * j + 2, hsl],
                                     e3, start=False, stop=True)
                rec = rw.tile([1, TL], F32, tag="rec")
                nc.vector.reciprocal(rec, den)
                dbc = st.tile([128, TL], F32, tag="dbc")
                nc.gpsimd.partition_broadcast(dbc, rec)
                nc.vector.tensor_mul(a2[h // 2][:, h % 2, :], ao, dbc)

            # ---------------- proj + residual, LN2 ----------------
            x2pre = [cp.tile([128, 2, TL], BF16, tag=f"y{i+2}", name=f"x2pre{i}") for i in range(2)]
            for et in range(4):
                pp = ps.tile([128, TL], F32, tag="mm")
                for kk in range(4):
                    nc.tensor.matmul(pp, wp_sb[:, kk, et * 128:(et + 1) * 128],
                                     a2[kk // 2][:, kk % 2, :],
                                     start=kk == 0, stop=kk == 3)
                nc.vector.tensor_add(x2pre[et // 2][:, et % 2, :],
                                     x1[:, et, WIN:WIN + TL], pp)
            x2 = cp.tile([128, 4, TL], BF16, tag="ywin")
            layer_norm(lambda et: x2pre[et // 2][:, et % 2, :], 1,
                       lambda et: x2[:, et, :], TL, chunk=256)

            # ---------------- FFN (ffn LN affine folded into W1/b1) -------
            xf = cp.tile([128, 4, TL], BF16, tag="x1")
            layer_norm(lambda et: x2[:, et, :], 2, lambda et: xf[:, et, :], TL,
                       affine=False, chunk=256)

            # token-split (tb halves) so x3/LN4/out of tb0 overlap FFN of tb1
            hg = [cp.tile([128, T], BF16, tag=f"u{i}", name=f"hga{i}") for i in range(4)] + \
                 [cp.tile([128, T], BF16, tag=f"um{i}", name=f"hgb{i}") for i in range(4)]
            x3 = cp.tile([128, 4, TL], BF16, tag="winT")
            outt = cp.tile([128, 4, TL], BF16, tag="ywin")
            for ft in range(16):
                hp = ps.tile([128, TL], F32, tag="mm")
                for kk in range(4):
                    nc.tensor.matmul(hp, w1_sb[:, kk, ft * 128:(ft + 1) * 128],
                                     xf[:, kk, :], start=kk == 0, stop=kk == 3)
                nc.scalar.activation(
                    hg[ft // 2][:, (ft % 2) * 512:(ft % 2) * 512 + 512],
                    hp, AF.Gelu, bias=b1_sb[:, ft:ft + 1])
            for tb in range(2):
                ts = slice(tb * 256, tb * 256 + 256)
                ops = [ps.tile([128, 256], F32, tag="mm", name=f"op0_{tb}"),
                       ps.tile([128, 256], F32, tag="mm", name=f"op1_{tb}"),
                       ps.tile([128, 256], F32, tag="scr12", name=f"op2_{tb}"),
                       ps.tile([128, 256], F32, tag="scr12", name=f"op3_{tb}")]
                for kk in range(16):
                    for et in range(4):
                        nc.tensor.matmul(
                            ops[et], w2_sb[:, kk, et * 128:(et + 1) * 128],
                            hg[kk // 2][:, (kk % 2) * 512 + tb * 256:
                                        (kk % 2) * 512 + tb * 256 + 256],
                            start=kk == 0, stop=kk == 15)
                for et in range(4):
                    nc.vector.scalar_tensor_tensor(
                        x3[:, et, ts], ops[et], b2_sb[:, et:et + 1],
                        x2[:, et, ts], ALU.add, ALU.add)
                layer_norm(lambda et: x3[:, et, ts], 3,
                           lambda et: outt[:, et, ts], 256, chunk=256)
                nc.sync.dma_start(out_d[:, :, ts], outt[:, :, ts])

    nc.compile()
    return nc


def _host_inputs(x, W_in, W_state, glru_g, glru_b, Wq, Wk, Wv, Wp, attn_g,
                 attn_b, ffn_g, ffn_b, W1, b1, W2, b2, out_g, out_b):
    f32 = np.float32
    bf = ml_dtypes.bfloat16
    cb = lambda a: np.ascontiguousarray(np.asarray(a, f32), dtype=None).astype(bf)
    ct = lambda a: np.ascontiguousarray(a, dtype=f32)
    W_in = np.asarray(W_in, f32)
    W_state = np.asarray(W_state, f32)
    Wc = W_state @ W_in[D:]                      # cand = x @ Wc.T
    W1 = np.asarray(W1, f32)
    ffn_g = np.asarray(ffn_g, f32)
    ffn_b = np.asarray(ffn_b, f32)
    W1g = W1 * ffn_g[None, :]
    b1f = np.asarray(b1, f32) + W1 @ ffn_b
    # per-feature LN affine columns: [p, g/b, ln_idx, et]
    gs = np.stack([glru_g, attn_g, np.ones(D, f32), out_g]).astype(f32)
    bs = np.stack([glru_b, attn_b, np.zeros(D, f32), out_b]).astype(f32)
    lncol = np.stack([gs, bs]).reshape(2, 4, 4, 128).transpose(3, 0, 1, 2)
    consts = np.concatenate(
        [b1f.reshape(FFD // 128, 128).T,
         np.asarray(b2, f32).reshape(D // 128, 128).T,
         lncol.reshape(128, 32)], axis=1)
    shared = {
        "w_in1T": cb(W_in[:D].T), "wcT": cb(Wc.T),
        "wqT": cb(np.asarray(Wq, f32).T), "wkT": cb(np.asarray(Wk, f32).T),
        "wvT": cb(np.asarray(Wv, f32).T), "wpT": cb(np.asarray(Wp, f32).T),
        "w1T": cb(W1g.T), "w2T": cb(np.asarray(W2, f32).T),
        "consts": ct(consts),
        "ones": np.ones((128, WT), bf),
    }
    in_maps = []
    xf32 = np.asarray(x, f32)
    for core in range(NCORES):
        b, half = core // 2, core % 2
        h0 = half * TL
        m = dict(shared)
        # local GLRU slab: global cols [h0-WIN-WARM, h0+TL+WIN), zero-padded
        start = h0 - WIN - WARM
        xloc = np.zeros((D, TG), f32)
        lo, hi = max(0, start), min(T, start + TG)
        xloc[:, lo - start:hi - start] = xf32[b].T[:, lo:hi]
        m["xt"] = xloc.astype(bf)
        # masks: window rows r (t' = h0 + 256j + r - 16), cols c (t = h0+256j+c)
        m12 = np.zeros((128, 2, 2, 256), f32)
        m3 = np.zeros((32, 2, 256), f32)
        c = np.arange(256)
        for j in range(2):
            for piece in range(3):
                r = np.arange(128 if piece < 2 else 32) + 128 * piece
                tpg = h0 + 256 * j + r - 16
                band = (np.abs(r[:, None] - c[None, :] - 16) <= 16) \
                    & (tpg[:, None] >= 0) & (tpg[:, None] < T)
                if piece < 2:
                    m12[:, j, piece, :] = band
                else:
                    m3[:, j, :] = band
        m["mask12"], m["mask3"] = m12.astype(bf), m3.astype(bf)
        in_maps.append(m)
    return in_maps


def kernel(**inputs):
    if "nc" not in _CACHE:
        _CACHE["nc"] = _build_nc()
    nc = _CACHE["nc"]
    in_maps = _host_inputs(**inputs)
    res = run_bass_kernel_spmd(nc, in_maps, core_ids=list(range(NCORES)),
                               **_CACHE.get("run_kwargs", {}))
    _CACHE["last_result"] = res
    out = np.empty((B, T, D), np.float32)
    for core in range(NCORES):
        b, half = core // 2, core % 2
        o = np.asarray(res.results[core]["outp"], np.float32)  # [128, 4, TL]
        out[b, half * TL:(half + 1) * TL, :] = \
            o.transpose(1, 0, 2).reshape(D, TL).T
    return out
